# revision 78
# baseline (speedup 1.0000x reference)
"""FP8 semi-sparse (2:4) activation linear — Trainium2 Bass/Tile kernel, v2.

Reference semantics:
  Wq, W_scale = rowwise fp8(e4m3fn) quant of weight      [N, K]
  Xq, X_scale = rowwise fp8(e4m3fn) quant of x           [M, K]
  Xsp         = 2:4 sparsify of Xq (keep 2 largest |.| per group of 4,
                ties -> earlier index)
  out         = (Xsp @ Wq^T) * X_scale * W_scale^T  -> bf16

v2 design (vs the v1 data-parallel kernel):
  * 2D core grid 4x2: core c=(mg*2+ng) gets x rows [mg*2048,+2048) and W rows
    [ng*2048,+2048), computes the [2048, 2048] out block.  Halves the
    replicated W-quant work and the per-core HBM traffic vs pure DP.
  * W^T fp8 (deinterleaved DoubleRow planes) is SBUF-resident (64KB/part);
    quantized per 128-row tile with SBUF->SBUF XBAR transposes (no DRAM
    round trip), interleaved with the first 4 X tiles.
  * X path per m-tile: f32 load (SWDGE ring, half-row stagger) -> DVE amax
    -> ACT fp8 quant -> 2:4 select -> SBUF->SBUF u16-pair transpose ->
    ACT deinterleave -> staged to DRAM, decoupling the X pipeline from
    W^T readiness; MM blocks reload it (prefetched one block ahead).
  * 2:4 compares run on contiguous bf16 byte-planes (ACT extracts them;
    sign-stripped fp8 -> bf16 is exact and monotone), heavy SBUF overlays
    keep the scratch at 28KB/partition.
  * MM: per-m block of 4 n-slice psums, t-outer for lhsT locality; DVE
    epilogue mult (f32, psum-direct) + ACT bf16 cast trail by one X tile
    so the in-order DVE queue never stalls on the PE.
  * TRN fp8e4 (max 240) vs OCP e4m3fn (max 448): quantize at HALF scale
    (g = 224/amax), fold the 4x into the output scale constant.
"""

import numpy as np

import concourse.bass as bass
import concourse.mybir as mybir
import concourse.tile as tile
from concourse import bacc
from concourse.bass_utils import run_bass_kernel_spmd

P = 128
M_FULL, K_FULL, N_FULL = 8192, 4096, 4096
NCORES = 8
MG, NG = 4, 2
M_CORE = M_FULL // MG    # 2048
N_CORE = N_FULL // NG    # 2048
N_SLICE = 512

F32 = mybir.dt.float32
BF16 = mybir.dt.bfloat16
FP8 = mybir.dt.float8e4
U16 = mybir.dt.uint16

AX = mybir.AxisListType.X
OP = mybir.AluOpType
AF = mybir.ActivationFunctionType

# out = acc' * amax_w * (amax_x * 4/448^2); acc' is the matmul of halved values
SX_CONST = float(np.float32(4.0 / (448.0 * 448.0)))


U32 = mybir.dt.uint32


def build_nc(m_core=M_CORE, k=K_FULL, n=N_CORE) -> bass.Bass:
    assert m_core % P == 0 and k % (2 * P) == 0 and n % N_SLICE == 0
    m_tiles = m_core // P          # 16
    kp_tiles = k // (2 * P)        # 16 packed k-pair tiles
    n_slices = n // N_SLICE        # 4 (also W bands)
    w_tiles = n // P               # 16
    wt_per_band = w_tiles // n_slices  # 4
    groups = k // 4
    kh = k // 2                    # half-row length (f32 load halves)

    nc = bacc.Bacc()
    x = nc.declare_dram_parameter("x", [m_core, k], F32, isOutput=False)
    w = nc.declare_dram_parameter("weight", [n, k], F32, isOutput=False)
    out = nc.declare_dram_parameter("out", [m_core, n], BF16, isOutput=True)

    with tile.TileContext(nc) as tc:
        with (
            tc.tile_pool(name="dram", bufs=1, space="DRAM") as dpool,
            tc.tile_pool(name="per", bufs=1) as perpool,
            tc.tile_pool(name="wld", bufs=2) as wldpool,
            tc.tile_pool(name="wq8", bufs=1) as wqpool,
            tc.tile_pool(name="wtp", bufs=1) as wtppool,
            tc.tile_pool(name="xld", bufs=2) as xldpool,
            tc.tile_pool(name="xu8", bufs=2) as u8pool,
            tc.tile_pool(name="cmp", bufs=1) as cpool,
            tc.tile_pool(name="xsT", bufs=2) as xspTpool,
            tc.tile_pool(name="xrl", bufs=2) as xrlpool,
            tc.tile_pool(name="sml", bufs=4) as spool,
            tc.tile_pool(name="ep", bufs=2) as eppool,
            tc.tile_pool(name="ps", bufs=8, space="PSUM") as pspool,
        ):
            xspT_dram = dpool.tile([m_tiles, P, kp_tiles, 2, P], FP8)
            wamax_dram = dpool.tile([n], F32)

            # persistent SBUF
            wk2 = perpool.tile([P, kp_tiles, 2, n], FP8)       # 64KB/part
            swb = perpool.tile([P, n], F32)                    # 8KB/part
            sx4 = perpool.tile([P, m_tiles], F32)

            xspT = {}  # m -> [P, kp_tiles, 2, P] fp8 tile

            # ---------------- W path ----------------
            kq = k // 4

            def _load_amax_quarters(dma, tile_, src_row, tagp):
                # quarter-row staggered load: the first reduce starts after
                # 512KB lands instead of the whole 2MB tile.
                for qi in range(4):
                    dma(tile_[:, kq * qi : kq * (qi + 1)],
                        src_row[:, kq * qi : kq * (qi + 1)])
                a0 = spool.tile([P, 1], F32, tag=f"{tagp}a0", name="a0")
                amax = spool.tile([P, 1], F32, tag=f"{tagp}am", name="am")
                nc.vector.tensor_reduce(
                    a0, tile_[:, :kq], axis=AX, op=OP.max,
                    apply_absolute_value=True,
                )
                for qi in range(1, 4):
                    nc.vector.tensor_reduce(
                        amax, tile_[:, kq * qi : kq * (qi + 1)], axis=AX,
                        op=OP.max, apply_absolute_value=True,
                    )
                    nc.vector.tensor_tensor(
                        a0 if qi < 3 else amax, amax, a0, op=OP.max
                    )
                return amax

            def w_quant(j):
                wt = wldpool.tile([P, k], F32, tag="wt")
                amax = _load_amax_quarters(
                    nc.scalar.dma_start, wt, w[P * j : P * (j + 1), :], "w"
                )
                g = spool.tile([P, 1], F32, tag="wg")
                nc.vector.reciprocal(g, amax)
                nc.vector.tensor_scalar_mul(g, g, 224.0)
                u8 = wqpool.tile([P, k], FP8, tag="wu8")
                nc.scalar.activation(u8, wt, AF.Copy, scale=g)
                nc.sync.dma_start(wamax_dram[P * j : P * (j + 1)], amax)
                # SBUF->SBUF transpose [128, 2048]u16 -> [128, 16, 128] and
                # deinterleave straight into this tile's wk2 column block.
                wtp = wtppool.tile([P, kp_tiles, P], U16, tag="wtp")
                nc.sync.dma_start_transpose(wtp, u8.bitcast(U16))
                pk = wtp.bitcast(FP8).rearrange("p t (r o) -> p t r o", o=2)
                for o in range(2):
                    nc.scalar.activation(
                        wk2[:, :, o, P * j : P * (j + 1)], pk[:, :, :, o], AF.Copy
                    )

            def swb_load():
                nc.sync.dma_start(
                    swb, wamax_dram.unsqueeze(0).to_broadcast([P, n])
                )

            # ---------------- X path ----------------
            def x_tile(mt):
                xt = xldpool.tile([P, k], F32, tag="xt")
                amax = _load_amax_quarters(
                    nc.gpsimd.dma_start, xt, x[P * mt : P * (mt + 1), :], "x"
                )
                nc.vector.tensor_scalar_mul(sx4[:, mt : mt + 1], amax, SX_CONST)
                g = spool.tile([P, 1], F32, tag="xg")
                nc.vector.reciprocal(g, amax)
                nc.vector.tensor_scalar_mul(g, g, 224.0)
                u8 = u8pool.tile([P, k], FP8, tag="xu8")
                nc.scalar.activation(u8, xt, AF.Copy, scale=g)

                # ---- 2:4 selection (DVE compares on bf16 planes) ----
                # SBUF-overlaid scratch: magmask holds mag then (later) the
                # byte-mask; scr holds the e0..e3 bf16 planes then the kk
                # flags; b6mt holds the 6 pairwise compares then mtmp;
                # stile holds the s partial sums then the masked xsp.
                magmask = cpool.tile([P, k // 2], U16, tag="magmask")
                scr = cpool.tile([P, k], BF16, tag="scr")
                b6mt = cpool.tile([P, 6, groups], BF16, tag="b6mt")
                stile = cpool.tile([P, k // 2], U16, tag="stile")

                mag = magmask
                nc.vector.tensor_scalar(
                    mag, u8.bitcast(U16), 0x7F7F, None, op0=OP.bitwise_and
                )
                # ACT extracts the 4 byte-planes as contiguous bf16 (the
                # sign-stripped fp8 -> bf16 cast is exact and monotone, so
                # integer byte compares become bf16 compares).
                mview = mag.bitcast(FP8).rearrange("p (g f) -> p g f", f=4)
                ev = scr.rearrange("p (a g) -> p a g", a=4)
                for i_ in range(4):
                    nc.scalar.activation(ev[:, i_, :], mview[:, :, i_], AF.Copy)
                e = {i_: ev[:, i_, :] for i_ in range(4)}

                b6 = b6mt
                pairs = [(0, 1), (0, 2), (0, 3), (1, 2), (1, 3), (2, 3)]
                bidx = {}
                for pi, (i, jj) in enumerate(pairs):
                    nc.vector.tensor_tensor(b6[:, pi, :], e[i], e[jj], op=OP.is_ge)
                    bidx[(i, jj)] = pi

                def b(i, jj):
                    return b6[:, bidx[(i, jj)], :]

                kk = scr.rearrange("p (a g) -> p a g", a=4)
                s = stile.bitcast(BF16).rearrange("p (a g) -> p a g", a=2)
                nc.vector.tensor_tensor(s[:, 0, :], b(0, 1), b(0, 2), op=OP.add)
                nc.vector.tensor_tensor(s[:, 0, :], s[:, 0, :], b(0, 3), op=OP.add)
                nc.vector.tensor_scalar(kk[:, 0, :], s[:, 0, :], 2.0, None, op0=OP.is_ge)
                nc.vector.tensor_tensor(s[:, 1, :], b(1, 2), b(1, 3), op=OP.add)
                nc.vector.tensor_tensor(s[:, 1, :], s[:, 1, :], b(0, 1), op=OP.subtract)
                nc.vector.tensor_scalar(kk[:, 1, :], s[:, 1, :], 1.0, None, op0=OP.is_ge)
                nc.vector.tensor_tensor(s[:, 0, :], b(2, 3), b(0, 2), op=OP.subtract)
                nc.vector.tensor_tensor(s[:, 0, :], s[:, 0, :], b(1, 2), op=OP.subtract)
                nc.vector.tensor_scalar(kk[:, 2, :], s[:, 0, :], 0.0, None, op0=OP.is_ge)
                nc.vector.tensor_tensor(s[:, 1, :], b(0, 3), b(1, 3), op=OP.add)
                nc.vector.tensor_tensor(s[:, 1, :], s[:, 1, :], b(2, 3), op=OP.add)
                nc.vector.tensor_scalar(kk[:, 3, :], s[:, 1, :], 1.0, None, op0=OP.is_le)

                mtmp = b6mt.bitcast(BF16).rearrange("p a g -> p a g")[:, :2, :]
                nc.vector.tensor_scalar_mul(mtmp[:, 0, :], kk[:, 0, :], 255.0)
                nc.vector.tensor_scalar_mul(mtmp[:, 1, :], kk[:, 2, :], 255.0)
                mask = magmask
                mv = mask.rearrange("p (g t) -> p g t", t=2)
                nc.vector.scalar_tensor_tensor(
                    mv[:, :, 0], kk[:, 1, :], 65280.0, mtmp[:, 0, :],
                    op0=OP.mult, op1=OP.add,
                )
                nc.vector.scalar_tensor_tensor(
                    mv[:, :, 1], kk[:, 3, :], 65280.0, mtmp[:, 1, :],
                    op0=OP.mult, op1=OP.add,
                )
                xsp = stile
                nc.vector.tensor_tensor(xsp, u8.bitcast(U16), mask, op=OP.bitwise_and)

                # SBUF->SBUF transpose [128, 2048]u16 -> [128, 16, 128];
                # output overlays u8 (fully consumed by the AND above), so the
                # cmp scratch frees without waiting on the deint.
                xtp = u8.bitcast(U16).rearrange("p (t r) -> p t r", t=kp_tiles)
                nc.sync.dma_start_transpose(xtp, xsp)
                xq = xspTpool.tile([P, kp_tiles, 2, P], FP8, tag="xq", name="xq")
                pk = xtp.bitcast(FP8).rearrange("p t (r o) -> p t r o", o=2)
                nc.scalar.activation(xq[:, :, 0, :], pk[:, :, :, 0], AF.Copy)
                nc.scalar.activation(xq[:, :, 1, :], pk[:, :, :, 1], AF.Copy)
                # stage to DRAM: decouples the X pipeline from band readiness
                nc.sync.dma_start(xspT_dram[mt], xq)

            # ---------------- MM pair-block (4 n-slices of one m-tile) ------
            xrls = {}

            def mm_reload(mt):
                xq = xrlpool.tile([P, kp_tiles, 2, P], FP8, tag="xrl", name="xrl")
                nc.sync.dma_start(xq, xspT_dram[mt])
                xrls[mt] = xq

            def mm_block(mt):
                xq = xrls.pop(mt)
                pss = [
                    pspool.tile([P, N_SLICE], F32, tag="ps", name=f"ps{mt}_{ns}")
                    for ns in range(n_slices)
                ]
                # t-outer so the stationary lhsT is loaded once per k-tile and
                # streamed against all 4 n-slices (4 MMs per LDWEIGHTS).
                for t in range(kp_tiles):
                    for ns in range(n_slices):
                        nc.tensor.matmul(
                            pss[ns],
                            lhsT=xq[:, t, :, :],
                            rhs=wk2[:, t, :, N_SLICE * ns : N_SLICE * (ns + 1)],
                            perf_mode=mybir.MatmulPerfMode.DoubleRow,
                            start=(t == 0),
                            stop=(t == kp_tiles - 1),
                        )
                for ns in range(n_slices):
                    ps = pss[ns]
                    ob = eppool.tile([P, N_SLICE], F32, tag="ob", name="ob")
                    nc.vector.tensor_tensor(
                        ob, ps, swb[:, N_SLICE * ns : N_SLICE * (ns + 1)], op=OP.mult
                    )
                    ob2 = eppool.tile([P, N_SLICE], BF16, tag="ob2", name="ob2")
                    nc.scalar.activation(ob2, ob, AF.Copy, scale=sx4[:, mt : mt + 1])
                    nc.sync.dma_start(
                        out[
                            P * mt : P * (mt + 1),
                            N_SLICE * ns : N_SLICE * (ns + 1),
                        ],
                        ob2,
                    )

            # ---------------- emission schedule ----------------
            # Opening: X0..X7 interleaved with 2 W tiles each (DVE alternates
            # 2:4 and W amax; ACT alternates casts; DMA streams loads).
            # MM phase: per-m pair-blocks (reload from DRAM staging) trail the
            # remaining X tiles by one so DVE epilogue mults never stall.
            wi = 0
            for mt in range(4):
                x_tile(mt)
                for _ in range(4):
                    w_quant(wi)
                    wi += 1
            swb_load()
            mm_reload(0)
            for mt in range(m_tiles):
                if 4 + mt < m_tiles:
                    x_tile(4 + mt)
                if mt + 1 < m_tiles:
                    mm_reload(mt + 1)
                mm_block(mt)

    return nc


_NC = None


def make_in_maps(x: np.ndarray, weight: np.ndarray) -> list[dict]:
    x = np.ascontiguousarray(x, dtype=np.float32)
    weight = np.ascontiguousarray(weight, dtype=np.float32)
    in_maps = []
    for c in range(NCORES):
        mg, ng = c // NG, c % NG
        in_maps.append(
            {
                "x": x[mg * M_CORE : (mg + 1) * M_CORE],
                "weight": weight[ng * N_CORE : (ng + 1) * N_CORE],
            }
        )
    return in_maps


def assemble_out(results: list[dict]) -> np.ndarray:
    rows = []
    for mg in range(MG):
        blocks = [results[mg * NG + ng]["out"] for ng in range(NG)]
        rows.append(np.concatenate(blocks, axis=1))
    return np.concatenate(rows, axis=0)


def kernel(x: np.ndarray, weight: np.ndarray) -> np.ndarray:
    global _NC
    if _NC is None:
        _NC = build_nc()
        _NC.finalize()
    res = run_bass_kernel_spmd(_NC, make_in_maps(x, weight), list(range(NCORES)))
    return assemble_out(res.results)
